# revision 3
# baseline (speedup 1.0000x reference)
"""Gaussian-kernel attention (out = x + alpha * exp(-r_sigma*d2(x_i,x_j)) @ x)
for B=4, T=4096, C=64 on 8 trn2 NeuronCores.

Sharding: core = b*2 + h handles batch b, query rows [h*2048, (h+1)*2048).
Each core receives x[b] ROTATED so its own query rows come first
(xrot = roll(x[b], -h*2048, axis=0)); key order is a permutation (the sum
over keys is permutation-invariant, so results are unchanged).  The host
also stages xrot^T (f32) so the kernel needs no on-device transposes.

Per-core algorithm (flash-attention style, K never hits HBM):
  stage 1:  S[s, t] = 2*g - sq_t  via one bf16 matmul with augmented
            contraction dim 65:
              A (65, T):    rows 0:64 = x^T, row 64 = ones
              R (65, ROWS): rows 0:64 = 2*x^T, row 64 = -sq_t
  exp:      K = exp(r_sigma*S + bias_s) on ScalarE, with the free affine
            of the ACTIVATE instruction carrying scale = r_sigma (per-
            partition AP) and bias_s = -r_sigma*sq_s (per-key AP).
            => K = exp(-r_sigma * (sq_s + sq_t - 2 g)) = exp(-r_sigma*d2)
  stage 2:  OT[c, t] += (alpha*x_chunk)^T @ K_chunk  (PSUM f32 accum;
            the two query halves go to PSUM partition groups 0:64 and
            64:128 via column tiling so OT fits in 2 PSUM banks)
  epilogue: res = x^T + OT  (DVE add; host transposes back)

At the actual operating point (r_sigma = 0) scale and bias are exactly 0,
so K = exp(0) = 1 exactly in any dtype; bf16 only rounds x once in
stage 2 (~3e-4 scale-relative output error).  The reference's clamp of
d2 at 0 only suppresses ~1e-6-scale rounding noise and is skipped.
"""

import numpy as np

B, T, C = 4, 4096, 64
NCORES = 8
ROWS = T // 2        # query rows per core
TB = 1024            # t-block width (one exp tile; 2 PSUM banks)
SC = 128             # s-chunk (keys per inner step)
NSC = T // SC        # 32
NTB = ROWS // TB     # 2
MMN = 512            # max matmul free dim (one PSUM bank of f32)
AUG = 65             # augmented contraction dim
WARM_MM = 26         # dense warmup matmuls to lift the PE HAM clock gate

_CACHE = {}


def _build_program():
    from contextlib import ExitStack

    import concourse.bass as bass  # noqa: F401
    import concourse.mybir as mybir
    import concourse.tile as tile
    from concourse import bacc

    f32 = mybir.dt.float32
    bf16 = mybir.dt.bfloat16
    Exp = mybir.ActivationFunctionType.Exp

    nc = bacc.Bacc(None, target_bir_lowering=False)
    xf = nc.dram_tensor("xf", (T, C), f32, kind="ExternalInput")
    xtf = nc.dram_tensor("xtf", (C, T), f32, kind="ExternalInput")
    rsig = nc.dram_tensor("rsig", (1, 1), f32, kind="ExternalInput")
    alp = nc.dram_tensor("alp", (1, 1), f32, kind="ExternalInput")
    out = nc.dram_tensor("out_ct", (2 * C, TB), f32, kind="ExternalOutput")

    with ExitStack() as ctx:
        tc = ctx.enter_context(tile.TileContext(nc))
        cp = ctx.enter_context(tc.tile_pool(name="const", bufs=1))

        # ---- warmup fodder + input loads ----
        wj = cp.tile([128, MMN], bf16)
        nc.vector.memset(wj, 0.0)

        # x chunked: xf_sb[p, si*C + c] = x[si*128 + p, c]
        xf_sb = cp.tile([128, NSC * C], f32)
        nc.sync.dma_start(
            xf_sb.rearrange("p (n c) -> p n c", c=C),
            xf.rearrange("(n p) c -> p n c", p=128),
        )
        xtf_sb = cp.tile([64, T], f32)       # x^T exact
        nc.sync.dma_start(xtf_sb[:, 0:ROWS], xtf[:, 0:ROWS])
        nc.sync.dma_start(xtf_sb[:, ROWS:T], xtf[:, ROWS:T])
        rsig_sb = cp.tile([1, 1], f32)
        nc.sync.dma_start(rsig_sb[:], rsig[:])
        alp_sb = cp.tile([1, 1], f32)
        nc.sync.dma_start(alp_sb[:], alp[:])

        ones_row = cp.tile([1, 128], f32)
        nc.vector.memset(ones_row, 1.0)
        ones_c64 = cp.tile([64, 1], bf16)
        nc.vector.memset(ones_c64, 1.0)

        # ---- stage-1 operands ----
        A_sb = cp.tile([AUG, T], bf16)       # [x^T; ones]
        R_sb = cp.tile([AUG, ROWS], bf16)    # [2*x^T; -sq_t]
        xa_sb = cp.tile([128, NSC * C], bf16)   # alpha * x (stage-2 weights)
        xsqT = cp.tile([64, ROWS], bf16)     # (x^T)^2 for the -sq_t row
        stage_nsq = cp.tile([1, ROWS], bf16)
        xsqn = cp.tile([128, NSC * C], f32)  # x*x natural layout
        sqn = cp.tile([128, NSC], f32)       # |x_s|^2 per key
        nrsq = cp.tile([128, NSC], f32)      # -r_sigma*|x_s|^2 (exp bias)
        rb_sb = cp.tile([128, 1], f32)       # r_sigma broadcast (exp scale)
        nrb_sb = cp.tile([128, 1], f32)      # -r_sigma broadcast
        ab_sb = cp.tile([128, 1], f32)       # alpha broadcast

        nc.vector.memset(A_sb[64:65, :], 1.0)

        with tc.tile_pool(name="pre", bufs=1, space="PSUM") as pre:
            # PE warmup: dense back-to-back matmuls while DMAs land.
            wp = pre.tile([128, MMN], f32)
            for _ in range(WARM_MM):
                nc.tensor.matmul(wp, wj[:, 0:128], wj, start=True, stop=True)

            # broadcast scalars across partitions (tiny matmuls)
            rb_ps = pre.tile([128, 1], f32)
            nc.tensor.matmul(rb_ps, ones_row, rsig_sb[:], start=True, stop=True)
            al_ps = pre.tile([128, 1], f32)
            nc.tensor.matmul(al_ps, ones_row, alp_sb[:], start=True, stop=True)
            nc.vector.tensor_copy(rb_sb, rb_ps)
            nc.vector.tensor_scalar_mul(nrb_sb, rb_ps, -1.0)
            nc.vector.tensor_copy(ab_sb, al_ps)

            # A rows 0:64 = x^T (bf16 cast, DVE); R rows 0:64 = 2*x^T (ACT)
            for g in range(2):
                gs = slice(g * ROWS, (g + 1) * ROWS)
                nc.vector.tensor_copy(A_sb[0:64, gs], xtf_sb[:, gs])
            for g in range(2):
                gs = slice(g * TB, (g + 1) * TB)
                nc.scalar.mul(R_sb[0:64, gs], xtf_sb[:, gs], 2.0)

            # -sq_t row: square (DVE), partition-reduce (4 PE matmuls),
            # negate (DVE), then SBUF->SBUF DMA into R row 64.
            sq_ps = pre.tile([1, ROWS], f32)
            for h in range(ROWS // MMN):
                hs = slice(h * MMN, (h + 1) * MMN)
                nc.vector.tensor_mul(xsqT[:, hs], xtf_sb[:, hs], xtf_sb[:, hs])
                nc.tensor.matmul(sq_ps[0:1, hs], ones_c64, xsqT[:, hs],
                                 start=True, stop=True)
                nc.vector.tensor_scalar_mul(stage_nsq[0:1, hs], sq_ps[0:1, hs],
                                            -1.0)
                nc.sync.dma_start(R_sb[64:65, hs], stage_nsq[0:1, hs])

            # exp bias chain: |x_s|^2 in natural layout, scaled by -r
            nc.vector.tensor_scalar_mul(xa_sb, xf_sb, ab_sb)
            nc.vector.tensor_mul(xsqn, xf_sb, xf_sb)
            nc.vector.tensor_reduce(
                sqn, xsqn.rearrange("p (n c) -> p n c", c=C),
                axis=mybir.AxisListType.X, op=mybir.AluOpType.add,
            )
            nc.vector.tensor_scalar_mul(nrsq, sqn, nrb_sb)

        # ---- main loop (ACT-bound: one 1024-wide exp per half-chunk) ----
        with (
            tc.tile_pool(name="spool", bufs=3, space="PSUM") as spool,
            tc.tile_pool(name="opool", bufs=1, space="PSUM") as opool,
            tc.tile_pool(name="kpool", bufs=3) as kpool,
        ):
            ot = opool.tile([128, TB], f32)  # [0:64]=tb0, [64:128]=tb1
            for si in range(NSC):
                asl = slice(si * SC, (si + 1) * SC)
                csl = slice(si * C, (si + 1) * C)
                for tb in range(NTB):
                    s_ps = spool.tile([128, TB], f32)
                    for h in range(TB // MMN):
                        qs = slice(tb * TB + h * MMN, tb * TB + (h + 1) * MMN)
                        nc.tensor.matmul(
                            s_ps[:, h * MMN:(h + 1) * MMN],
                            A_sb[:, asl], R_sb[:, qs],
                            start=True, stop=True,
                        )
                    k_sb = kpool.tile([128, TB], bf16)
                    nc.scalar.activation(
                        k_sb, s_ps, Exp,
                        bias=nrsq[:, si:si + 1], scale=rb_sb[:, 0:1],
                    )
                    psl = slice(64 * tb, 64 * (tb + 1))
                    for h in range(TB // MMN):
                        hs = slice(h * MMN, (h + 1) * MMN)
                        nc.tensor.matmul(
                            ot[psl, hs], xa_sb[:, csl], k_sb[:, hs],
                            start=(si == 0), stop=(si == NSC - 1),
                        )

            # ---- epilogue: res = x^T + OT ----
            exT = cp.tile([128, TB], f32)   # x^T packed as [tb*64+c, q]
            nc.sync.dma_start(exT[0:64, :], xtf[:, 0:TB])
            nc.sync.dma_start(exT[64:128, :], xtf[:, TB:ROWS])
            res = cp.tile([128, TB], f32)
            nc.vector.tensor_add(res, exT, ot)
            nc.sync.dma_start(out[:], res[:])

    return nc


def _get_program():
    if "nc" not in _CACHE:
        nc = _build_program()
        if not nc.is_finalized():
            nc.finalize()  # runs Bacc legalization (wait splitting, reg alloc)
        _CACHE["nc"] = nc
    return _CACHE["nc"]


def _make_in_maps(x, r_sigma, alpha):
    x = np.asarray(x, np.float32)
    rs = np.float32(np.asarray(r_sigma).reshape(())).reshape(1, 1)
    al = np.float32(np.asarray(alpha).reshape(())).reshape(1, 1)
    in_maps = []
    for core in range(NCORES):
        b, h = divmod(core, 2)
        xrot = np.roll(x[b], -h * ROWS, axis=0)
        in_maps.append({
            "xf": np.ascontiguousarray(xrot),
            "xtf": np.ascontiguousarray(xrot.T),
            "rsig": np.ascontiguousarray(rs),
            "alp": np.ascontiguousarray(al),
        })
    return in_maps


def kernel_with_results(x, r_sigma, alpha, trace=False):
    from concourse.bass_utils import run_bass_kernel_spmd

    nc = _get_program()
    res = run_bass_kernel_spmd(
        nc, _make_in_maps(x, r_sigma, alpha), core_ids=list(range(NCORES)),
        trace=trace,
    )
    out = np.empty((B, T, C), np.float32)
    for core in range(NCORES):
        b, h = divmod(core, 2)
        r = res.results[core]["out_ct"].reshape(NTB, C, TB)
        out[b, h * ROWS:(h + 1) * ROWS] = (
            r.transpose(0, 2, 1).reshape(ROWS, C)
        )
    return out, res


def kernel(x, r_sigma, alpha):
    out, _ = kernel_with_results(x, r_sigma, alpha)
    return out


# revision 4
# speedup vs baseline: 1.3460x; 1.3460x over previous
"""Gaussian-kernel attention (out = x + alpha * exp(-r_sigma*d2(x_i,x_j)) @ x)
for B=4, T=4096, C=64 on 8 trn2 NeuronCores.

Sharding: core = b*2 + h handles batch b, query rows [h*2048, (h+1)*2048).
Each core receives x[b] ROTATED so its own query rows come first
(xrot = roll(x[b], -h*2048, axis=0)); key order is a permutation (the sum
over keys is permutation-invariant, so results are unchanged).  The host
also stages xrot^T (f32) so the kernel needs no on-device transposes.

The kernel exp factorizes:  K = exp(-r*d2) = es_s * E * w_t  with
  E[s,t] = exp(2r * <x_s, x_t>)   (the only T x T term)
  es_s   = exp(-r*|x_s|^2)        folded into the stage-2 weights
  w_t    = exp(-r*|x_t|^2)        folded into the epilogue
so the hot loop is:
  stage 1:  S = A^T R, contraction 64:  A = x^T, R = (2r*x)^T  (bf16).
            A/R are duplicated into both SBUF partition halves so
            consecutive key chunks run CONCURRENTLY as row-tiled
            matmuls (PE is hard-capped at 1.2 GHz here; 32x32 tile
            concurrency is the only way past 1 col/cycle).
  exp:      E = exp(S) on ScalarE, immediate scale/bias (AP operands
            cost +330ns/instr), 1024-wide from PSUM.
  stage 2:  P[tb*64+c, q] = (alpha*es*x_chunk)^T @ E_chunk, the two
            query halves col-tiled into PSUM partition groups so one
            (128,1024) partial serves both; DVE accumulates partials
            into an SBUF f32 accumulator (frees all 8 PSUM banks for
            a 4-slot s_ps/partial pipeline).
  epilogue: res = x^T + acc * W  (W = w_t broadcast via tiny matmuls).

At the actual operating point (r_sigma = 0): R = 0 exactly, S = 0, E = 1,
es = w = 1 exactly, so only the single bf16 rounding of x in stage 2
matters (~3e-4 scale-relative output error).  The reference's clamp of
d2 at 0 only suppresses ~1e-6-scale rounding noise and is skipped.
"""

import numpy as np

B, T, C = 4, 4096, 64
NCORES = 8
ROWS = T // 2        # query rows per core
TB = 1024            # t-block width (one exp tile; 2 PSUM banks)
SC = 128             # s-chunk (keys per inner step)
NSC = T // SC        # 32
NTB = ROWS // TB     # 2
MMN = 512            # max matmul free dim (one PSUM bank of f32)

_CACHE = {}


def _build_program():
    from contextlib import ExitStack

    import concourse.bass as bass  # noqa: F401
    import concourse.mybir as mybir
    import concourse.tile as tile
    from concourse import bacc

    f32 = mybir.dt.float32
    bf16 = mybir.dt.bfloat16
    Exp = mybir.ActivationFunctionType.Exp

    nc = bacc.Bacc(None, target_bir_lowering=False)
    xf = nc.dram_tensor("xf", (T, C), f32, kind="ExternalInput")
    xtf = nc.dram_tensor("xtf", (C, T), f32, kind="ExternalInput")
    rsig = nc.dram_tensor("rsig", (1, 1), f32, kind="ExternalInput")
    alp = nc.dram_tensor("alp", (1, 1), f32, kind="ExternalInput")
    out = nc.dram_tensor("out_ct", (2 * C, TB), f32, kind="ExternalOutput")

    with ExitStack() as ctx:
        tc = ctx.enter_context(tile.TileContext(nc))
        cp = ctx.enter_context(tc.tile_pool(name="const", bufs=1))

        # ---- input loads ----
        # x chunked: xf_sb[p, si*C + c] = x[si*128 + p, c]
        xf_sb = cp.tile([128, NSC * C], f32)
        nc.sync.dma_start(
            xf_sb.rearrange("p (n c) -> p n c", c=C),
            xf.rearrange("(n p) c -> p n c", p=128),
        )
        # x^T duplicated into both partition halves (for row tiling)
        xtfd = cp.tile([128, T], f32)
        nc.sync.dma_start(xtfd[0:64, :], xtf[:])
        nc.sync.dma_start(xtfd[64:128, :], xtf[:])
        rsig_sb = cp.tile([1, 1], f32)
        nc.sync.dma_start(rsig_sb[:], rsig[:])
        alp_sb = cp.tile([1, 1], f32)
        nc.sync.dma_start(alp_sb[:], alp[:])

        ones_row = cp.tile([1, 128], f32)
        nc.vector.memset(ones_row, 1.0)
        ones_rb = cp.tile([1, 128], bf16)
        nc.vector.memset(ones_rb, 1.0)
        ones_c64 = cp.tile([64, 1], bf16)
        nc.vector.memset(ones_c64, 1.0)

        # ---- derived operands ----
        A_big = cp.tile([128, T], bf16)      # x^T (both halves)
        R_big = cp.tile([128, ROWS], bf16)   # 2r * x^T (both halves)
        xa_sb = cp.tile([128, NSC * C], bf16)   # alpha*es*x (stage-2 weights)
        xsqn = cp.tile([128, NSC * C], f32)  # x*x natural layout
        sqn = cp.tile([128, NSC], f32)       # |x_s|^2 per key
        nrsq = cp.tile([128, NSC], f32)      # -r*|x_s|^2
        es_sb = cp.tile([128, NSC], f32)     # exp(-r*|x_s|^2)
        aes_sb = cp.tile([128, NSC], f32)    # alpha * es
        xsqT = cp.tile([64, ROWS], bf16)     # (x^T)^2 for w_t
        nsqT = cp.tile([1, ROWS], f32)       # -r*|x_t|^2
        wexp = cp.tile([1, ROWS], bf16)      # w_t = exp(-r*|x_t|^2)
        rb_sb = cp.tile([128, 1], f32)       # r broadcast
        rb2_sb = cp.tile([128, 1], f32)      # 2r broadcast
        nrb_sb = cp.tile([128, 1], f32)      # -r broadcast
        ab_sb = cp.tile([128, 1], f32)       # alpha broadcast
        acc = cp.tile([128, TB], f32)        # SBUF accumulator for stage 2
        nc.vector.memset(acc, 0.0)

        with tc.tile_pool(name="pre", bufs=1, space="PSUM") as pre:
            # broadcast scalars across partitions (tiny matmuls)
            rb_ps = pre.tile([128, 1], f32)
            nc.tensor.matmul(rb_ps, ones_row, rsig_sb[:], start=True, stop=True)
            al_ps = pre.tile([128, 1], f32)
            nc.tensor.matmul(al_ps, ones_row, alp_sb[:], start=True, stop=True)
            nc.vector.tensor_copy(rb_sb, rb_ps)
            nc.vector.tensor_scalar_mul(rb2_sb, rb_ps, 2.0)
            nc.vector.tensor_scalar_mul(nrb_sb, rb_ps, -1.0)
            nc.vector.tensor_copy(ab_sb, al_ps)

            # A = x^T cast, R = 2r*x^T (both partition halves at once)
            nc.vector.tensor_copy(A_big, xtfd)
            nc.vector.tensor_scalar_mul(R_big, xtfd[:, 0:ROWS], rb2_sb)

            # s-side: |x_s|^2 -> -r*sq -> es = exp(-r*sq) -> aes = alpha*es
            nc.vector.tensor_mul(xsqn, xf_sb, xf_sb)
            nc.vector.tensor_reduce(
                sqn, xsqn.rearrange("p (n c) -> p n c", c=C),
                axis=mybir.AxisListType.X, op=mybir.AluOpType.add,
            )
            nc.vector.tensor_scalar_mul(nrsq, sqn, nrb_sb)
            nc.scalar.activation(es_sb, nrsq, Exp)  # also pre-loads Exp table
            nc.vector.tensor_scalar_mul(aes_sb, es_sb, ab_sb)
            for si in range(NSC):
                csl = slice(si * C, (si + 1) * C)
                nc.vector.tensor_scalar_mul(
                    xa_sb[:, csl], xf_sb[:, csl], aes_sb[:, si:si + 1])

            # t-side: |x_t|^2 via ones-matmul partition reduce -> w_t
            sq_ps = pre.tile([1, ROWS], f32)
            for h in range(ROWS // MMN):
                hs = slice(h * MMN, (h + 1) * MMN)
                nc.vector.tensor_mul(xsqT[:, hs], xtfd[0:64, hs],
                                     xtfd[0:64, hs])
                nc.tensor.matmul(sq_ps[0:1, hs], ones_c64, xsqT[:, hs],
                                 start=True, stop=True)
                nc.vector.tensor_scalar_mul(nsqT[0:1, hs], sq_ps[0:1, hs],
                                            nrb_sb[0:1, :])
            nc.scalar.activation(wexp, nsqT, Exp)

        # ---- main loop: chunk pairs, row-tiled stage 1, col-tiled stage 2
        with (
            tc.tile_pool(name="mm", bufs=4, space="PSUM") as mm,
            tc.tile_pool(name="kpool", bufs=4) as kpool,
        ):
            for p in range(NSC // 2):
                c0, c1 = 2 * p, 2 * p + 1
                a0 = slice(c0 * SC, (c0 + 1) * SC)
                a1 = slice(c1 * SC, (c1 + 1) * SC)
                x0 = slice(c0 * C, (c0 + 1) * C)
                x1 = slice(c1 * C, (c1 + 1) * C)
                for j in range(NTB):
                    tba, tbb = j, 1 - j
                    sA = mm.tile([128, TB], f32, tag="s_ps")
                    sB = mm.tile([128, TB], f32, tag="s_ps")
                    # interleave row groups so the PE runs both streams
                    for h in range(TB // MMN):
                        hs = slice(h * MMN, (h + 1) * MMN)
                        qa = slice(tba * TB + h * MMN, tba * TB + (h + 1) * MMN)
                        qb = slice(tbb * TB + h * MMN, tbb * TB + (h + 1) * MMN)
                        nc.tensor.matmul(sA[:, hs], A_big[0:64, a0],
                                         R_big[0:64, qa], start=True, stop=True)
                        nc.tensor.matmul(sB[:, hs], A_big[64:128, a1],
                                         R_big[64:128, qb], start=True,
                                         stop=True)
                    kA = kpool.tile([128, TB], bf16, tag="k")
                    nc.scalar.activation(kA, sA, Exp)
                    kB = kpool.tile([128, TB], bf16, tag="k")
                    nc.scalar.activation(kB, sB, Exp)
                    # stage 2: one col-tiled partial for both query halves
                    pt = mm.tile([128, TB], f32, tag="s_ps")
                    for h in range(TB // MMN):
                        hs = slice(h * MMN, (h + 1) * MMN)
                        pa = slice(64 * tba, 64 * tba + 64)
                        pb = slice(64 * tbb, 64 * tbb + 64)
                        nc.tensor.matmul(pt[pa, hs], xa_sb[:, x0], kA[:, hs],
                                         start=True, stop=True)
                        nc.tensor.matmul(pt[pb, hs], xa_sb[:, x1], kB[:, hs],
                                         start=True, stop=True)
                    nc.vector.tensor_add(acc, acc, pt)

            # ---- epilogue: res = x^T + acc * W ----
            W_ps = mm.tile([128, TB], f32, tag="s_ps")
            for g in range(2):
                for h in range(TB // MMN):
                    hs = slice(h * MMN, (h + 1) * MMN)
                    ws = slice(g * TB + h * MMN, g * TB + (h + 1) * MMN)
                    nc.tensor.matmul(W_ps[64 * g:64 * g + 64, hs],
                                     ones_rb[:, 0:64], wexp[0:1, ws],
                                     start=True, stop=True)
            exT = cp.tile([128, TB], f32)   # x^T packed as [tb*64+c, q]
            nc.sync.dma_start(exT[0:64, :], xtf[:, 0:TB])
            nc.sync.dma_start(exT[64:128, :], xtf[:, TB:ROWS])
            res = cp.tile([128, TB], f32)
            nc.vector.tensor_mul(res, acc, W_ps)
            nc.vector.tensor_add(res, res, exT)
            nc.sync.dma_start(out[:], res[:])

    return nc


def _get_program():
    if "nc" not in _CACHE:
        nc = _build_program()
        if not nc.is_finalized():
            nc.finalize()  # runs Bacc legalization (wait splitting, reg alloc)
        _CACHE["nc"] = nc
    return _CACHE["nc"]


def _make_in_maps(x, r_sigma, alpha):
    x = np.asarray(x, np.float32)
    rs = np.float32(np.asarray(r_sigma).reshape(())).reshape(1, 1)
    al = np.float32(np.asarray(alpha).reshape(())).reshape(1, 1)
    in_maps = []
    for core in range(NCORES):
        b, h = divmod(core, 2)
        xrot = np.roll(x[b], -h * ROWS, axis=0)
        in_maps.append({
            "xf": np.ascontiguousarray(xrot),
            "xtf": np.ascontiguousarray(xrot.T),
            "rsig": np.ascontiguousarray(rs),
            "alp": np.ascontiguousarray(al),
        })
    return in_maps


def kernel_with_results(x, r_sigma, alpha, trace=False):
    from concourse.bass_utils import run_bass_kernel_spmd

    nc = _get_program()
    res = run_bass_kernel_spmd(
        nc, _make_in_maps(x, r_sigma, alpha), core_ids=list(range(NCORES)),
        trace=trace,
    )
    out = np.empty((B, T, C), np.float32)
    for core in range(NCORES):
        b, h = divmod(core, 2)
        r = res.results[core]["out_ct"].reshape(NTB, C, TB)
        out[b, h * ROWS:(h + 1) * ROWS] = (
            r.transpose(0, 2, 1).reshape(ROWS, C)
        )
    return out, res


def kernel(x, r_sigma, alpha):
    out, _ = kernel_with_results(x, r_sigma, alpha)
    return out
